# revision 2
# baseline (speedup 1.0000x reference)
"""Trainium2 Bass kernel for batched multi-head attention (8 NeuronCores).

Problem: x[8,1024,1024], Wq[1024,1024], bq[1024], Wkv[1024,2048], bkv[2048]
  q = x@Wq+bq ; k,v = split(x@Wkv+bkv, 2) ;
  out = softmax(q k^T / sqrt(64), over keys) @ v        (16 heads, d=64)

Sharding: data-parallel over batch — one batch element per NeuronCore; weights
replicated. No collectives needed; outputs are stacked on the host.

Per-core kernel design (all bf16 matmuls, fp32 PSUM accumulation):
  - x^T loaded straight from HBM via transposing DMA (bf16).
  - q^T, k^T (c on partitions) and v (tokens on partitions) computed on the
    PE; q/k biases fused into the PSUM->SBUF copy (per-partition bias add on
    VectorE); the v bias is applied post-attention, exactly:
    softmax rows sum to 1 =>  P(V + 1 bv^T)/Z = PV/Z + bv.
  - s^T = k^T.T @ q^T puts softmax keys on the PARTITION dim, so attention
    probabilities come out already transposed for the att@v matmul — no
    on-chip transposes of [1024,1024] probability tiles are ever needed.
  - exp on ScalarE with the 1/sqrt(d) scale fused into the activation;
    max-subtraction is skipped (logits are bounded ~|4| by construction,
    exp is overflow-safe in fp32, and softmax is shift-invariant).
  - v carries an extra ones-column, so the att@v matmul emits the softmax
    denominator Z alongside the unnormalized output; normalization + v-bias
    are one fused VectorE op: out = (o * 1/Z) + bv.
  - Weight/x tiles are split per 128-row chunk so matmuls start as soon as
    the first chunks land; K/Q projections for a column block are emitted
    just before that block's two heads, keeping ScalarE (the 134us exp
    floor) fed while the PE works through projections.
"""

from contextlib import ExitStack

import numpy as np
import ml_dtypes

import concourse.bass as bass
import concourse.mybir as mybir
import concourse.tile as tile
from concourse import bacc
from concourse.bass_utils import run_bass_kernel_spmd

P = 128
N = 1024
C = 1024
H = 16
D = 64
NCH = N // P
B = 8
SCALE = D ** -0.5
F32 = mybir.dt.float32
BF16 = mybir.dt.bfloat16
EXP = mybir.ActivationFunctionType.Exp
MULT = mybir.AluOpType.mult
ADD = mybir.AluOpType.add


def _build():
    nc = bacc.Bacc("TRN2")
    x = nc.dram_tensor("x", [N, C], BF16, kind="ExternalInput")
    wq = nc.dram_tensor("wq", [C, C], BF16, kind="ExternalInput")
    bq = nc.dram_tensor("bq", [C], F32, kind="ExternalInput")
    wkv = nc.dram_tensor("wkv", [C, 2 * C], BF16, kind="ExternalInput")
    bkv = nc.dram_tensor("bkv", [2 * C], F32, kind="ExternalInput")
    out = nc.dram_tensor("out", [N, C], F32, kind="ExternalOutput")

    with ExitStack() as ctx:
        tc = ctx.enter_context(tile.TileContext(nc))
        persist = ctx.enter_context(tc.tile_pool(name="persist", bufs=1))

        xT_t = [persist.tile([P, N], BF16, tag=f"xT{i}", name=f"xT{i}")
                for i in range(NCH)]
        wq_t = [persist.tile([P, C], BF16, tag=f"wq{i}", name=f"wq{i}")
                for i in range(NCH)]
        wkv_t = [persist.tile([P, 2 * C], BF16, tag=f"wkv{i}", name=f"wkv{i}")
                 for i in range(NCH)]
        qT_t = [persist.tile([P, N], BF16, tag=f"qT{i}", name=f"qT{i}")
                for i in range(NCH)]
        kT_t = [persist.tile([P, N], BF16, tag=f"kT{i}", name=f"kT{i}")
                for i in range(NCH)]
        v_sb = persist.tile([P, NCH, H, D + 1], BF16, tag="v")
        out_t = [persist.tile([P, C], F32, tag=f"out{i}", name=f"out{i}")
                 for i in range(NCH)]
        bq_sb = persist.tile([P, NCH], F32, tag="bq")
        bk_sb = persist.tile([P, NCH], F32, tag="bk")
        bv_bc = persist.tile([P, C], F32, tag="bv")
        scratch = persist.tile([P, 512], BF16, tag="scratch")

        pt_pool = ctx.enter_context(tc.tile_pool(name="pt", bufs=2))
        rz_pool = ctx.enter_context(tc.tile_pool(name="rz", bufs=8))
        proj_ps = ctx.enter_context(tc.tile_pool(name="proj_ps", bufs=2, space="PSUM"))
        s_ps = ctx.enter_context(tc.tile_pool(name="s_ps", bufs=2, space="PSUM"))
        o_ps = ctx.enter_context(tc.tile_pool(name="o_ps", bufs=2, space="PSUM"))

        # PE warmup (p-state ramp) on a zeroed scratch tile, no downstream deps
        nc.vector.memset(scratch[:], 0.0)
        wps = proj_ps.tile([P, 512], F32, tag="ps", name="wups")
        for _ in range(10):
            nc.tensor.matmul(wps[:], scratch[:, 0:P], scratch[:],
                             start=True, stop=True)

        # input DMAs, finest-grained, first-needed first
        for cc in range(NCH):
            nc.sync.dma_start(out=xT_t[cc][:], in_=x.ap()[:, cc * P:(cc + 1) * P],
                              transpose=True)
        for kc in range(NCH):
            nc.scalar.dma_start(out=wkv_t[kc][:],
                                in_=wkv.ap()[kc * P:(kc + 1) * P, :])
        nc.sync.dma_start(out=bk_sb[:],
                          in_=bkv.ap()[0:C].rearrange("(cc p) -> p cc", p=P))
        for kc in range(NCH):
            nc.scalar.dma_start(out=wq_t[kc][:],
                                in_=wq.ap()[kc * P:(kc + 1) * P, :])
        nc.sync.dma_start(out=bq_sb[:], in_=bq.ap().rearrange("(cc p) -> p cc", p=P))
        bv_row = bkv.ap()[C:2 * C]
        nc.sync.dma_start(
            out=bv_bc[:],
            in_=bass.AP(tensor=bv_row.tensor, offset=bv_row.offset,
                        ap=[[0, P]] + list(bv_row.ap)),
        )

        def proj_group(lhs_fn, rhs_fn):
            ps = proj_ps.tile([P, 512], F32, tag="ps", name="ps")
            for kc in range(NCH):
                nc.tensor.matmul(ps[:], lhs_fn(kc), rhs_fn(kc),
                                 start=(kc == 0), stop=(kc == NCH - 1))
            return ps

        def v_proj(mc, ch):
            ps = proj_group(
                lambda kc: xT_t[kc][:, mc * P:(mc + 1) * P],
                lambda kc: wkv_t[kc][:, C + ch * 512:C + (ch + 1) * 512],
            )
            nc.vector.tensor_copy(
                v_sb[:, mc, ch * 8:(ch + 1) * 8, 0:D],
                ps[:].rearrange("p (h d) -> p h d", d=D),
            )

        nc.vector.memset(v_sb[:, :, :, D], 1.0)

        for cc in range(NCH):
            for nh in range(2):
                ps = proj_group(
                    lambda kc: wkv_t[kc][:, cc * P:(cc + 1) * P],
                    lambda kc: xT_t[kc][:, nh * 512:(nh + 1) * 512],
                )
                nc.vector.tensor_scalar_add(
                    kT_t[cc][:, nh * 512:(nh + 1) * 512], ps[:],
                    bk_sb[:, cc:cc + 1])
        for mc in range(NCH):
            v_proj(mc, 0)

        for cc in range(NCH):
            for nh in range(2):
                ps = proj_group(
                    lambda kc: wq_t[kc][:, cc * P:(cc + 1) * P],
                    lambda kc: xT_t[kc][:, nh * 512:(nh + 1) * 512],
                )
                nc.vector.tensor_scalar_add(
                    qT_t[cc][:, nh * 512:(nh + 1) * 512], ps[:],
                    bq_sb[:, cc:cc + 1])

            for hr in range(2):
                h = 2 * cc + hr
                prow = hr * D
                pt = pt_pool.tile([P, NCH, N], BF16, tag="pt", name="pt")
                for mc in range(NCH):
                    s = s_ps.tile([P, N], F32, tag="s", name="s")
                    for nh in range(2):
                        nc.tensor.matmul(
                            s[:, nh * 512:(nh + 1) * 512],
                            kT_t[cc][prow:prow + D, mc * P:(mc + 1) * P],
                            qT_t[cc][prow:prow + D, nh * 512:(nh + 1) * 512],
                            start=True, stop=True)
                    nc.scalar.activation(pt[:, mc, :], s[:], EXP, scale=SCALE)
                for ni in range(NCH):
                    o = o_ps.tile([P, D + 1], F32, tag="o", name="o")
                    for mc in range(NCH):
                        nc.tensor.matmul(
                            o[:], pt[:, mc, ni * P:(ni + 1) * P],
                            v_sb[:, mc, h, :],
                            start=(mc == 0), stop=(mc == NCH - 1))
                    rz = rz_pool.tile([P, 1], F32, tag="rz", name="rz")
                    nc.vector.reciprocal(rz[:], o[:, D:D + 1])
                    nc.vector.scalar_tensor_tensor(
                        out_t[ni][:, h * D:(h + 1) * D],
                        o[:, 0:D], rz[:], bv_bc[:, h * D:(h + 1) * D],
                        op0=MULT, op1=ADD)

            # V second half under the ScalarE-bound attention phase
            if cc == 0:
                for mc in range(NCH):
                    v_proj(mc, 1)

        for ni in range(NCH):
            eng = nc.sync if ni % 2 == 0 else nc.scalar
            eng.dma_start(out=out.ap()[ni * P:(ni + 1) * P, :], in_=out_t[ni][:])

    nc.finalize()
    return nc


_NC = None


def kernel(x, Wq, bq, Wkv, bkv):
    global _NC
    if _NC is None:
        _NC = _build()
    bf = ml_dtypes.bfloat16
    x = np.asarray(x)
    wq_b = np.asarray(Wq).astype(bf)
    wkv_b = np.asarray(Wkv).astype(bf)
    bq_f = np.asarray(bq).astype(np.float32)
    bkv_f = np.asarray(bkv).astype(np.float32)
    in_maps = [
        {"x": x[b].astype(bf), "wq": wq_b, "bq": bq_f, "wkv": wkv_b,
         "bkv": bkv_f}
        for b in range(B)
    ]
    res = run_bass_kernel_spmd(_NC, in_maps, core_ids=list(range(B)))
    return np.stack([res.results[b]["out"] for b in range(B)]).astype(np.float32)
